# revision 19
# baseline (speedup 1.0000x reference)
"""Trainium2 Bass kernel for causal multi-head attention + output projection.

Problem (hardcoded): x[4, 2048, 1024] fp32, 16 heads, head_dim 64, causal,
torch-Linear convention (y = x @ W.T), output projection with bias.

Sharding over 8 NeuronCores: batch (4) x head-group (2 groups of 8 heads).
Each core computes q/k/v for its 8 heads of its batch, causal attention in
the S^T layout (keys on partitions, queries on free dim; softmax denominators
produced by an appended ones-column in V), then the output projection.

Combine modes:
  - "a2a": on-device AllToAll per head swaps query-halves between the two
    cores of a batch so each core projects all 16 heads for its own 1024
    queries; outputs are disjoint rows, host just concatenates.
  - "hostsum": each core emits a partial projection over its 8 heads for all
    2048 queries; host sums the pair (bias folded into group-0's input).

All matmuls run as float32r (TF32-like, ~1.5e-4 rel err, 4x faster than fp32).
"""
import os
import sys
import types

import numpy as np

import concourse.bass as bass
import concourse.mybir as mybir
import concourse.tile as tile
from concourse import bacc, bass_utils

DT = mybir.dt.float32r
F32 = mybir.dt.float32
AF = mybir.ActivationFunctionType
OP = mybir.AluOpType

B, T, D = 4, 2048, 1024
H, HD = 16, 64
HG = 8          # heads per core
QH = T // 2     # query half
N_CORES = 8
SCALE = 1.0 / 8.0

MODE = os.environ.get("ATTN_KERNEL_MODE", "a2a")
ADT_NAME = os.environ.get("ATTN_DTYPE", "float32r")
ADT = getattr(mybir.dt, ADT_NAME)


# ---------------------------------------------------------------------------
# environment glue
# ---------------------------------------------------------------------------

def _install_ntff_hook():
    if 'antenv.axon_hooks' in sys.modules:
        return
    try:
        from trn_agent_boot.trn_boot import _ntff_profile_via_ctypes
        hook = _ntff_profile_via_ctypes('/opt/axon/libaxon_pjrt.so')
    except Exception:
        hook = None
    mod = types.ModuleType('antenv.axon_hooks')
    mod.get_axon_ntff_profile_hook = lambda: hook
    mod.set_axon_ntff_profile_hook = lambda h: None
    sys.modules['antenv.axon_hooks'] = mod


def _run_spmd(nc, in_maps, trace=False):
    from concourse.bass_interp import get_hw_module
    bass_utils.upload_artifacts = lambda tmpdir: tmpdir
    if trace:
        _install_ntff_hook()
    old_m = nc.m
    nc.m = get_hw_module(nc.m)
    try:
        return bass_utils.run_bass_kernel_spmd(
            nc, in_maps, core_ids=list(range(N_CORES)),
            trace=trace, trace_cores=[0] if trace else None,
        )
    finally:
        nc.m = old_m


# ---------------------------------------------------------------------------
# kernel program
# ---------------------------------------------------------------------------

def _qkv_phase(nc, tc, ctx, xT, wqT, wkT, wvT, vone, qT_sb, kT_sb, v_sb):
    """Compute q.T [512,2048], k.T [512,2048] and V' [2048, 8, 65] for this
    core's 8 heads. Contraction dim D lives on partitions; all operands fp32r."""
    xp = ctx.enter_context(tc.tile_pool(name="xph", bufs=16))
    wp = ctx.enter_context(tc.tile_pool(name="wph", bufs=12))
    ps = ctx.enter_context(tc.tile_pool(name="p2ps", bufs=2, space="PSUM"))

    xT_r = xT.rearrange("(ko ki) t -> ki ko t", ki=128)
    xh = [[None] * 8 for _ in range(2)]
    for kk in range(8):
        for half in range(2):
            t = xp.tile([128, QH], DT, tag="xh")
            nc.sync.dma_start(t[:], xT_r[:, kk, half * QH:(half + 1) * QH])
            xh[half][kk] = t

    def load_w(wT):
        parts = []
        wT_r = wT.rearrange("(ko ki) n -> ki ko n", ki=128)
        for kk in range(8):
            t = wp.tile([128, 512], DT, tag="w")
            nc.sync.dma_start(t[:], wT_r[:, kk])
            parts.append(t)
        return parts

    # k.T then V (both need full xT), then q.T half by half
    wk_sb = load_w(wkT)
    for m in range(4):
        for half in range(2):
            pt = ps.tile([128, QH], F32, tag="st")
            for nch in range(2):
                sl = slice(nch * 512, (nch + 1) * 512)
                for kk in range(8):
                    nc.tensor.matmul(
                        pt[:, sl],
                        lhsT=wk_sb[kk][:, m * 128:(m + 1) * 128],
                        rhs=xh[half][kk][:, sl],
                        start=(kk == 0), stop=(kk == 7))
            nc.vector.tensor_copy(kT_sb[:, m, half * QH:(half + 1) * QH], pt[:])

    wv_sb = load_w(wvT)
    nc.sync.dma_start(v_sb[:, :, :, 64],
                      vone.rearrange("p (a b) -> p a b", a=16))
    for m in range(16):
        pt = ps.tile([128, QH], F32, tag="st")
        for kk in range(8):
            nc.tensor.matmul(
                pt[:, 0:512],
                lhsT=xh[m // 8][kk][:, (m % 8) * 128:(m % 8 + 1) * 128],
                rhs=wv_sb[kk][:],
                start=(kk == 0), stop=(kk == 7))
        nc.vector.tensor_copy(
            v_sb[:, m, :, 0:64],
            pt[:, 0:512].rearrange("p (h d) -> p h d", h=HG))

    wq_sb = load_w(wqT)
    for half in range(2):
        for m in range(4):
            pt = ps.tile([128, QH], F32, tag="st")
            for nch in range(2):
                sl = slice(nch * 512, (nch + 1) * 512)
                for kk in range(8):
                    nc.tensor.matmul(
                        pt[:, sl],
                        lhsT=wq_sb[kk][:, m * 128:(m + 1) * 128],
                        rhs=xh[half][kk][:, sl],
                        start=(kk == 0), stop=(kk == 7))
            nc.vector.tensor_copy(qT_sb[:, m, half * QH:(half + 1) * QH], pt[:])


def _attend_pair(nc, p, qT_sb, kT_sb, v_sb, mask_sb, ps, es, snum, srec,
                 evict_cb, norm_cb):
    """Heads (2p, 2p+1) with their j-steps interleaved so the PE always has an
    independent S/AV matmul while the other head's exp runs on ACT. Rows 0..63
    of each accumulator are un-normalized O.T, row 64 the softmax denominators;
    normalization uses a reciprocal spread over 64 partitions via DRAM."""
    heads = (2 * p, 2 * p + 1)
    for qh in range(2):
        jmax = 8 * qh + 8
        o_ps = {h: ps.tile([65, QH], F32, tag="o", name=f"o{h}_{qh}")
                for h in heads}
        for j in range(jmax):
            qstart = max(QH * qh, 128 * j)
            n = QH * (qh + 1) - qstart
            coff = qstart - QH * qh
            e_sbs = {}
            for h in heads:
                pbase = 64 * (h % 2)
                sub = h // 2
                s_ps = ps.tile([128, QH], F32, tag="st", name=f"s{h}")
                for c in range(0, n, 512):
                    cn = min(512, n - c)
                    nc.tensor.matmul(
                        s_ps[:, c:c + cn],
                        lhsT=kT_sb[pbase:pbase + 64, sub, j * 128:(j + 1) * 128],
                        rhs=qT_sb[pbase:pbase + 64, sub,
                                  qstart + c:qstart + c + cn],
                        start=True, stop=True)
                e_sb = es.tile([128, QH], ADT, tag="es", name=f"e{h}")
                nc.scalar.activation(e_sb[:, 0:n], s_ps[:, 0:n], AF.Exp,
                                     scale=SCALE)
                if j >= 8 * qh:
                    nc.vector.tensor_tensor(
                        e_sb[:, 0:128], e_sb[:, 0:128], mask_sb[:], OP.mult)
                e_sbs[h] = e_sb
            for h in heads:
                c0 = coff
                while c0 < QH:
                    hi = min(QH, (c0 // 512 + 1) * 512)
                    nc.tensor.matmul(
                        o_ps[h][:, c0:hi],
                        lhsT=v_sb[:, j, h, :],
                        rhs=e_sbs[h][:, c0 - coff:hi - coff],
                        start=(j == 0), stop=(j == jmax - 1),
                        skip_group_check=True)
                    c0 = hi
        for h in heads:
            evict_cb(h, qh, o_ps[h])
            i = 4 * (h // 2) + 2 * (h % 2) + qh
            stmp = es.tile([1, QH], F32, tag="sr")
            nc.scalar.copy(stmp[:], o_ps[h][64:65, :])
            nc.sync.dma_start(snum[i:i + 1, :], stmp[:])
            st64 = es.tile([64, QH // 64], F32, tag="sp")
            nc.sync.dma_start(st64[:], snum[i].rearrange("(p f) -> p f", p=64))
            nc.vector.reciprocal(st64[:], st64[:])
            nc.sync.dma_start(srec[i].rearrange("(p f) -> p f", p=64), st64[:])
            bc = es.tile([128, QH], F32, tag="bc")
            nc.sync.dma_start(bc[:], srec[i][None, :].broadcast_to([128, QH]))
            norm_cb(h, qh, bc)


def build_nc(mode):
    nc = bacc.Bacc("TRN2", target_bir_lowering=False, debug=False,
                   enable_asserts=False, num_devices=N_CORES)
    xT = nc.dram_tensor("xT", [D, T], DT, kind="ExternalInput").ap()
    wqT = nc.dram_tensor("wqT", [D, 512], DT, kind="ExternalInput").ap()
    wkT = nc.dram_tensor("wkT", [D, 512], DT, kind="ExternalInput").ap()
    wvT = nc.dram_tensor("wvT", [D, 512], DT, kind="ExternalInput").ap()
    mask = nc.dram_tensor("mask", [128, 128], ADT, kind="ExternalInput").ap()
    ones = nc.dram_tensor("ones", [128, 128], DT, kind="ExternalInput").ap()
    vone = nc.dram_tensor("vone", [128, 128], ADT, kind="ExternalInput").ap()
    snum = nc.dram_tensor("snum", [16, QH], F32).ap()
    srec = nc.dram_tensor("srec", [16, QH], F32).ap()
    if mode == "a2a":
        wpT = nc.dram_tensor("wpT", [D, D], DT, kind="ExternalInput").ap()
        bias = nc.dram_tensor("bias", [1, D], DT, kind="ExternalInput").ap()
        y = nc.dram_tensor("y", [QH, D], F32, kind="ExternalOutput").ap()
    else:
        wpT = nc.dram_tensor("wpT", [512, D], DT, kind="ExternalInput").ap()
        bias = nc.dram_tensor("bias", [1, D], DT, kind="ExternalInput").ap()
        y = nc.dram_tensor("y", [T, D], F32, kind="ExternalOutput").ap()

    from contextlib import ExitStack
    with tile.TileContext(nc) as tc, ExitStack() as ctx:
        per = ctx.enter_context(tc.tile_pool(name="per", bufs=1))

        qT_sb = per.tile([128, 4, T], ADT, tag="qT")
        kT_sb = per.tile([128, 4, T], ADT, tag="kT")
        v_sb = per.tile([128, 16, HG, 65], ADT, tag="v")
        ones_sb = per.tile([128, 128], DT, tag="ones")
        mask_sb = per.tile([128, 128], ADT, tag="mask")
        bias_sb = per.tile([1, D], DT, tag="bias")
        nc.sync.dma_start(ones_sb[:], ones[:])
        nc.sync.dma_start(mask_sb[:], mask[:])
        nc.sync.dma_start(bias_sb[:], bias[:])

        with ExitStack() as p2:
            _qkv_phase(nc, tc, p2, xT, wqT, wkT, wvT, vone, qT_sb, kT_sb, v_sb)

        # O accumulator (lives from attention through projection)
        mid = ctx.enter_context(tc.tile_pool(name="mid", bufs=1))
        o_all = mid.tile([128, 8 if mode == "a2a" else 4,
                          QH if mode == "a2a" else T], DT, tag="oacc")
        wp_sb = None
        if mode != "a2a":
            wp_sb = mid.tile([128, 4, D], DT, tag="wp")
            nc.sync.dma_start(wp_sb[:],
                              wpT.rearrange("(ko ki) n -> ki ko n", ki=128))

        with ExitStack() as attn:
            ps = attn.enter_context(tc.tile_pool(name="aps", bufs=2, space="PSUM"))
            es = attn.enter_context(tc.tile_pool(name="es", bufs=3))

            if mode == "a2a":
                dram = attn.enter_context(
                    tc.tile_pool(name="dram", bufs=2, space="DRAM"))
                oh_pool = attn.enter_context(tc.tile_pool(name="oh", bufs=2))
                for p in range(HG // 2):
                    ohs = {h: oh_pool.tile([64, T], DT, tag="oh", name=f"oh{h}")
                           for h in (2 * p, 2 * p + 1)}

                    def evict_cb(h, qh, o_ps, ohs=ohs):
                        nc.vector.tensor_copy(
                            ohs[h][:, QH * qh:QH * (qh + 1)], o_ps[0:64, :])

                    def norm_cb(h, qh, bc, ohs=ohs):
                        sl_ap = ohs[h][:, QH * qh:QH * (qh + 1)]
                        nc.vector.tensor_tensor(sl_ap, sl_ap, bc[0:64, :],
                                                OP.mult)

                    _attend_pair(nc, p, qT_sb, kT_sb, v_sb, mask_sb,
                                 ps, es, snum, srec, evict_cb, norm_cb)

                    for h in (2 * p, 2 * p + 1):
                        oh_sb = ohs[h]
                        in_b = dram.tile([2, 64, QH], DT, tag="cin")
                        out_b = dram.tile([2, 64, QH], DT, tag="cout",
                                          addr_space="Shared")
                        for s in range(2):
                            nc.sync.dma_start(in_b[s],
                                              oh_sb[:, s * QH:(s + 1) * QH])
                        nc.gpsimd.collective_compute(
                            "AllToAll", OP.bypass,
                            replica_groups=[[0, 1], [2, 3], [4, 5], [6, 7]],
                            ins=[in_b[:]], outs=[out_b[:]],
                        )
                        for s in range(2):
                            gh = 8 * s + h
                            nc.sync.dma_start(
                                o_all[64 * (gh % 2):64 * (gh % 2) + 64,
                                      gh // 2, :],
                                out_b[s])
            else:
                def evict_cb(h, qh, o_ps):
                    nc.vector.tensor_copy(
                        o_all[64 * (h % 2):64 * (h % 2) + 64, h // 2,
                              QH * qh:QH * (qh + 1)],
                        o_ps[0:64, :])

                def norm_cb(h, qh, bc):
                    pb = 64 * (h % 2)
                    sl_ap = o_all[pb:pb + 64, h // 2, QH * qh:QH * (qh + 1)]
                    nc.vector.tensor_tensor(sl_ap, sl_ap, bc[pb:pb + 64, :],
                                            OP.mult)

                for p in range(HG // 2):
                    _attend_pair(nc, p, qT_sb, kT_sb, v_sb, mask_sb,
                                 ps, es, snum, srec, evict_cb, norm_cb)

        # projection
        with ExitStack() as proj:
            pps = proj.enter_context(tc.tile_pool(name="pps", bufs=2, space="PSUM"))
            wpool = proj.enter_context(tc.tile_pool(name="wpp", bufs=1))
            yo = proj.enter_context(tc.tile_pool(name="yo", bufs=3))
            n_kk = 8 if mode == "a2a" else 4
            n_m = 8 if mode == "a2a" else 16
            if wp_sb is None:
                wp_sb = wpool.tile([128, n_kk, D], DT, tag="wp")
                nc.sync.dma_start(
                    wp_sb[:], wpT.rearrange("(ko ki) n -> ki ko n", ki=128))
            for m in range(n_m):
                yp = pps.tile([128, D], F32, tag="yp")
                for nch in range(2):
                    sl = slice(nch * 512, (nch + 1) * 512)
                    for kk in range(n_kk):
                        nc.tensor.matmul(
                            yp[:, sl],
                            lhsT=o_all[:, kk, m * 128:(m + 1) * 128],
                            rhs=wp_sb[:, kk, sl],
                            start=(kk == 0), stop=False)
                    nc.tensor.matmul(yp[:, sl], lhsT=ones_sb[0:1, 0:128],
                                     rhs=bias_sb[0:1, sl], start=False, stop=True)
                y_sb = yo.tile([128, D], F32, tag="y")
                nc.vector.tensor_copy(y_sb[:], yp[:])
                nc.sync.dma_start(y[m * 128:(m + 1) * 128, :], y_sb[:])

    nc.compile()
    return nc


# ---------------------------------------------------------------------------
# host-side sharding + entry point
# ---------------------------------------------------------------------------

_NC_CACHE = {}


def _get_nc(mode):
    if mode not in _NC_CACHE:
        _NC_CACHE[mode] = build_nc(mode)
    return _NC_CACHE[mode]


def _make_in_maps(x, Wq, Wk, Wv, Wp, bp, mode):
    x = np.asarray(x, dtype=np.float32)
    Wq = np.asarray(Wq, dtype=np.float32)
    Wk = np.asarray(Wk, dtype=np.float32)
    Wv = np.asarray(Wv, dtype=np.float32)
    Wp = np.asarray(Wp, dtype=np.float32)
    bp = np.asarray(bp, dtype=np.float32)

    adt_np = mybir.dt.np(ADT)
    mask = np.zeros((128, 128), dtype=np.float32)
    k_idx = np.arange(128)[:, None]
    q_idx = np.arange(128)[None, :]
    mask[q_idx >= k_idx] = 1.0
    mask = mask.astype(adt_np)

    xTs = [np.ascontiguousarray(x[b].T) for b in range(B)]
    in_maps = []
    for c in range(N_CORES):
        b, g = c // 2, c % 2
        rows = slice(512 * g, 512 * (g + 1))
        m = {
            "xT": xTs[b],
            "wqT": np.ascontiguousarray(Wq[rows, :].T),
            "wkT": np.ascontiguousarray(Wk[rows, :].T),
            "wvT": np.ascontiguousarray(Wv[rows, :].T),
            "mask": mask,
            "ones": np.ones((128, 128), dtype=np.float32),
            "vone": np.ones((128, 128), dtype=adt_np),
        }
        if mode == "a2a":
            m["wpT"] = np.ascontiguousarray(Wp.T)
            m["bias"] = bp.reshape(1, D)
        else:
            m["wpT"] = np.ascontiguousarray(Wp[:, rows].T)
            m["bias"] = (bp if g == 0 else np.zeros_like(bp)).reshape(1, D)
        in_maps.append(m)
    return in_maps


def kernel(x, Wq, Wk, Wv, Wp, bp, _trace=False, _mode=None):
    mode = _mode or MODE
    nc = _get_nc(mode)
    in_maps = _make_in_maps(x, Wq, Wk, Wv, Wp, bp, mode)
    res = _run_spmd(nc, in_maps, trace=_trace)
    out = np.empty((B, T, D), dtype=np.float32)
    for b in range(B):
        if mode == "a2a":
            out[b, 0:QH] = res.results[2 * b]["y"]
            out[b, QH:T] = res.results[2 * b + 1]["y"]
        else:
            out[b] = res.results[2 * b]["y"] + res.results[2 * b + 1]["y"]
    if _trace:
        kernel.last_results = res
    return out


# revision 21
# speedup vs baseline: 1.1016x; 1.1016x over previous
"""Trainium2 Bass kernel for causal multi-head attention + output projection.

Problem (hardcoded): x[4, 2048, 1024] fp32, 16 heads, head_dim 64, causal,
torch-Linear convention (y = x @ W.T), output projection with bias.

Sharding over 8 NeuronCores: batch (4) x head-group (2 groups of 8 heads).
Each core computes q/k/v for its 8 heads of its batch, causal attention in
the S^T layout (keys on partitions, queries on free dim; softmax denominators
produced by an appended ones-column in V), then the output projection.

Combine modes:
  - "a2a": on-device AllToAll per head swaps query-halves between the two
    cores of a batch so each core projects all 16 heads for its own 1024
    queries; outputs are disjoint rows, host just concatenates.
  - "hostsum": each core emits a partial projection over its 8 heads for all
    2048 queries; host sums the pair (bias folded into group-0's input).

All matmuls run as float32r (TF32-like, ~1.5e-4 rel err, 4x faster than fp32).
"""
import os
import sys
import types

import numpy as np

import concourse.bass as bass
import concourse.mybir as mybir
import concourse.tile as tile
from concourse import bacc, bass_utils

DT = mybir.dt.float32r
F32 = mybir.dt.float32
AF = mybir.ActivationFunctionType
OP = mybir.AluOpType

B, T, D = 4, 2048, 1024
H, HD = 16, 64
HG = 8          # heads per core
QH = T // 2     # query half
N_CORES = 8
SCALE = 1.0 / 8.0

MODE = os.environ.get("ATTN_KERNEL_MODE", "a2a")
ADT_NAME = os.environ.get("ATTN_DTYPE", "float32r")
ADT = getattr(mybir.dt, ADT_NAME)


# ---------------------------------------------------------------------------
# environment glue
# ---------------------------------------------------------------------------

def _install_ntff_hook():
    if 'antenv.axon_hooks' in sys.modules:
        return
    try:
        from trn_agent_boot.trn_boot import _ntff_profile_via_ctypes
        hook = _ntff_profile_via_ctypes('/opt/axon/libaxon_pjrt.so')
    except Exception:
        hook = None
    mod = types.ModuleType('antenv.axon_hooks')
    mod.get_axon_ntff_profile_hook = lambda: hook
    mod.set_axon_ntff_profile_hook = lambda h: None
    sys.modules['antenv.axon_hooks'] = mod


def _run_spmd(nc, in_maps, trace=False):
    from concourse.bass_interp import get_hw_module
    bass_utils.upload_artifacts = lambda tmpdir: tmpdir
    if trace:
        _install_ntff_hook()
    old_m = nc.m
    nc.m = get_hw_module(nc.m)
    try:
        return bass_utils.run_bass_kernel_spmd(
            nc, in_maps, core_ids=list(range(N_CORES)),
            trace=trace, trace_cores=[0] if trace else None,
        )
    finally:
        nc.m = old_m


# ---------------------------------------------------------------------------
# kernel program
# ---------------------------------------------------------------------------

def _qkv_phase(nc, tc, ctx, xT, wqT, wkT, wvT, vone, qT_sb, kT_sb, v_sb):
    """Compute q.T [512,2048], k.T [512,2048] and V' [2048, 8, 65] for this
    core's 8 heads. Contraction dim D lives on partitions; all operands fp32r."""
    xp = ctx.enter_context(tc.tile_pool(name="xph", bufs=16))
    wp = ctx.enter_context(tc.tile_pool(name="wph", bufs=12))
    ps = ctx.enter_context(tc.tile_pool(name="p2ps", bufs=2, space="PSUM"))

    xT_r = xT.rearrange("(ko ki) t -> ki ko t", ki=128)
    xh = [[None] * 8 for _ in range(2)]
    for kk in range(8):
        for half in range(2):
            t = xp.tile([128, QH], DT, tag="xh")
            nc.sync.dma_start(t[:], xT_r[:, kk, half * QH:(half + 1) * QH])
            xh[half][kk] = t

    def load_w(wT):
        parts = []
        wT_r = wT.rearrange("(ko ki) n -> ki ko n", ki=128)
        for kk in range(8):
            t = wp.tile([128, 512], DT, tag="w")
            nc.sync.dma_start(t[:], wT_r[:, kk])
            parts.append(t)
        return parts

    # k.T then V (both need full xT), then q.T half by half
    wk_sb = load_w(wkT)
    for m in range(4):
        for half in range(2):
            pt = ps.tile([128, QH], F32, tag="st")
            for nch in range(2):
                sl = slice(nch * 512, (nch + 1) * 512)
                for kk in range(8):
                    nc.tensor.matmul(
                        pt[:, sl],
                        lhsT=wk_sb[kk][:, m * 128:(m + 1) * 128],
                        rhs=xh[half][kk][:, sl],
                        start=(kk == 0), stop=(kk == 7))
            nc.vector.tensor_copy(kT_sb[:, m, half * QH:(half + 1) * QH], pt[:])

    wv_sb = load_w(wvT)
    nc.sync.dma_start(v_sb[:, :, :, 64],
                      vone.rearrange("p (a b) -> p a b", a=16))
    for m in range(16):
        pt = ps.tile([128, QH], F32, tag="st")
        for kk in range(8):
            nc.tensor.matmul(
                pt[:, 0:512],
                lhsT=xh[m // 8][kk][:, (m % 8) * 128:(m % 8 + 1) * 128],
                rhs=wv_sb[kk][:],
                start=(kk == 0), stop=(kk == 7))
        nc.vector.tensor_copy(
            v_sb[:, m, :, 0:64],
            pt[:, 0:512].rearrange("p (h d) -> p h d", h=HG))

    wq_sb = load_w(wqT)
    for half in range(2):
        for m in range(4):
            pt = ps.tile([128, QH], F32, tag="st")
            for nch in range(2):
                sl = slice(nch * 512, (nch + 1) * 512)
                for kk in range(8):
                    nc.tensor.matmul(
                        pt[:, sl],
                        lhsT=wq_sb[kk][:, m * 128:(m + 1) * 128],
                        rhs=xh[half][kk][:, sl],
                        start=(kk == 0), stop=(kk == 7))
            nc.vector.tensor_copy(qT_sb[:, m, half * QH:(half + 1) * QH], pt[:])


def _attend_pair(nc, p, qT_sb, kT_sb, v_sb, mask_sb, ps, es, snum, srec,
                 evict_cb, norm_cb):
    """Heads (2p, 2p+1) with their j-steps interleaved so the PE always has an
    independent S/AV matmul while the other head's exp runs on ACT. Rows 0..63
    of each accumulator are un-normalized O.T, row 64 the softmax denominators;
    normalization uses a reciprocal spread over 64 partitions via DRAM."""
    heads = (2 * p, 2 * p + 1)
    for qh in range(2):
        jmax = 8 * qh + 8
        o_ps = {h: ps.tile([65, QH], F32, tag="o", name=f"o{h}_{qh}")
                for h in heads}
        for j in range(jmax):
            qstart = max(QH * qh, 128 * j)
            n = QH * (qh + 1) - qstart
            coff = qstart - QH * qh
            e_sbs = {}
            for h in heads:
                pbase = 64 * (h % 2)
                sub = h // 2
                s_ps = ps.tile([128, QH], F32, tag="st", name=f"s{h}")
                for c in range(0, n, 512):
                    cn = min(512, n - c)
                    nc.tensor.matmul(
                        s_ps[:, c:c + cn],
                        lhsT=kT_sb[pbase:pbase + 64, sub, j * 128:(j + 1) * 128],
                        rhs=qT_sb[pbase:pbase + 64, sub,
                                  qstart + c:qstart + c + cn],
                        start=True, stop=True)
                e_sb = es.tile([128, QH], ADT, tag="es", name=f"e{h}")
                nc.scalar.activation(e_sb[:, 0:n], s_ps[:, 0:n], AF.Exp,
                                     scale=SCALE)
                if j >= 8 * qh:
                    nc.vector.tensor_tensor(
                        e_sb[:, 0:128], e_sb[:, 0:128], mask_sb[:], OP.mult)
                e_sbs[h] = e_sb
            for h in heads:
                c0 = coff
                while c0 < QH:
                    hi = min(QH, (c0 // 512 + 1) * 512)
                    nc.tensor.matmul(
                        o_ps[h][:, c0:hi],
                        lhsT=v_sb[:, j, h, :],
                        rhs=e_sbs[h][:, c0 - coff:hi - coff],
                        start=(j == 0), stop=(j == jmax - 1),
                        skip_group_check=True)
                    c0 = hi
        for h in heads:
            evict_cb(h, qh, o_ps[h])
            i = 4 * (h // 2) + 2 * (h % 2) + qh
            stmp = es.tile([1, QH], F32, tag="sr")
            nc.scalar.copy(stmp[:], o_ps[h][64:65, :])
            nc.sync.dma_start(snum[i:i + 1, :], stmp[:])
            st64 = es.tile([64, QH // 64], F32, tag="sp")
            nc.sync.dma_start(st64[:], snum[i].rearrange("(p f) -> p f", p=64))
            nc.vector.reciprocal(st64[:], st64[:])
            nc.sync.dma_start(srec[i].rearrange("(p f) -> p f", p=64), st64[:])
            bc = es.tile([128, QH], F32, tag="bc")
            nc.sync.dma_start(bc[:], srec[i][None, :].broadcast_to([128, QH]))
            norm_cb(h, qh, bc)


def build_nc(mode):
    nc = bacc.Bacc("TRN2", target_bir_lowering=False, debug=False,
                   enable_asserts=False, num_devices=N_CORES)
    xT = nc.dram_tensor("xT", [D, T], DT, kind="ExternalInput").ap()
    wqT = nc.dram_tensor("wqT", [D, 512], DT, kind="ExternalInput").ap()
    wkT = nc.dram_tensor("wkT", [D, 512], DT, kind="ExternalInput").ap()
    wvT = nc.dram_tensor("wvT", [D, 512], DT, kind="ExternalInput").ap()
    mask = nc.dram_tensor("mask", [128, 128], ADT, kind="ExternalInput").ap()
    vone = nc.dram_tensor("vone", [128, 128], ADT, kind="ExternalInput").ap()
    snum = nc.dram_tensor("snum", [16, QH], F32).ap()
    srec = nc.dram_tensor("srec", [16, QH], F32).ap()
    if mode == "a2a":
        wpT = nc.dram_tensor("wpT", [D, D], DT, kind="ExternalInput").ap()
        bias = nc.dram_tensor("bias", [1, D], F32, kind="ExternalInput").ap()
        y = nc.dram_tensor("y", [QH, D], F32, kind="ExternalOutput").ap()
    else:
        wpT = nc.dram_tensor("wpT", [512, D], DT, kind="ExternalInput").ap()
        bias = nc.dram_tensor("bias", [1, D], F32, kind="ExternalInput").ap()
        y = nc.dram_tensor("y", [T, D], F32, kind="ExternalOutput").ap()

    from contextlib import ExitStack
    with tile.TileContext(nc) as tc, ExitStack() as ctx:
        per = ctx.enter_context(tc.tile_pool(name="per", bufs=1))

        qT_sb = per.tile([128, 4, T], ADT, tag="qT")
        kT_sb = per.tile([128, 4, T], ADT, tag="kT")
        v_sb = per.tile([128, 16, HG, 65], ADT, tag="v")
        mask_sb = per.tile([128, 128], ADT, tag="mask")

        nc.sync.dma_start(mask_sb[:], mask[:])
        bias_f = bias[0]

        with ExitStack() as p2:
            _qkv_phase(nc, tc, p2, xT, wqT, wkT, wvT, vone, qT_sb, kT_sb, v_sb)

        # O accumulator (lives from attention through projection)
        mid = ctx.enter_context(tc.tile_pool(name="mid", bufs=1))
        o_all = mid.tile([128, 8 if mode == "a2a" else 4,
                          QH if mode == "a2a" else T], DT, tag="oacc")
        wp_sb = None
        if mode != "a2a":
            wp_sb = mid.tile([128, 4, D], DT, tag="wp")
            nc.sync.dma_start(wp_sb[:],
                              wpT.rearrange("(ko ki) n -> ki ko n", ki=128))

        att_ps = ctx.enter_context(tc.tile_pool(name="aps", bufs=2, space="PSUM"))
        with ExitStack() as attn:
            ps = att_ps
            es = attn.enter_context(tc.tile_pool(name="es", bufs=3))

            if mode == "a2a":
                dram = attn.enter_context(
                    tc.tile_pool(name="dram", bufs=2, space="DRAM"))
                oh_pool = attn.enter_context(tc.tile_pool(name="oh", bufs=2))
                for p in range(HG // 2):
                    ohs = {h: oh_pool.tile([64, T], DT, tag="oh", name=f"oh{h}")
                           for h in (2 * p, 2 * p + 1)}

                    def evict_cb(h, qh, o_ps, ohs=ohs):
                        nc.vector.tensor_copy(
                            ohs[h][:, QH * qh:QH * (qh + 1)], o_ps[0:64, :])

                    def norm_cb(h, qh, bc, ohs=ohs):
                        sl_ap = ohs[h][:, QH * qh:QH * (qh + 1)]
                        nc.vector.tensor_tensor(sl_ap, sl_ap, bc[0:64, :],
                                                OP.mult)

                    _attend_pair(nc, p, qT_sb, kT_sb, v_sb, mask_sb,
                                 ps, es, snum, srec, evict_cb, norm_cb)

                    for h in (2 * p, 2 * p + 1):
                        oh_sb = ohs[h]
                        in_b = dram.tile([2, 64, QH], DT, tag="cin")
                        out_b = dram.tile([2, 64, QH], DT, tag="cout",
                                          addr_space="Shared")
                        for s in range(2):
                            nc.sync.dma_start(in_b[s],
                                              oh_sb[:, s * QH:(s + 1) * QH])
                        nc.gpsimd.collective_compute(
                            "AllToAll", OP.bypass,
                            replica_groups=[[0, 1], [2, 3], [4, 5], [6, 7]],
                            ins=[in_b[:]], outs=[out_b[:]],
                        )
                        for s in range(2):
                            gh = 8 * s + h
                            nc.sync.dma_start(
                                o_all[64 * (gh % 2):64 * (gh % 2) + 64,
                                      gh // 2, :],
                                out_b[s])
            else:
                def evict_cb(h, qh, o_ps):
                    nc.vector.tensor_copy(
                        o_all[64 * (h % 2):64 * (h % 2) + 64, h // 2,
                              QH * qh:QH * (qh + 1)],
                        o_ps[0:64, :])

                def norm_cb(h, qh, bc):
                    pb = 64 * (h % 2)
                    sl_ap = o_all[pb:pb + 64, h // 2, QH * qh:QH * (qh + 1)]
                    nc.vector.tensor_tensor(sl_ap, sl_ap, bc[pb:pb + 64, :],
                                            OP.mult)

                for p in range(HG // 2):
                    _attend_pair(nc, p, qT_sb, kT_sb, v_sb, mask_sb,
                                 ps, es, snum, srec, evict_cb, norm_cb)

        # projection (psum from the attention pool so the scheduler can
        # overlap early m-tiles with the last head's attention)
        n_kk = 8 if mode == "a2a" else 4
        n_m = 8 if mode == "a2a" else 16
        if wp_sb is None:
            wp_sb = mid.tile([128, n_kk, D], DT, tag="wp")
            nc.sync.dma_start(
                wp_sb[:], wpT.rearrange("(ko ki) n -> ki ko n", ki=128))
        bias_bc = mid.tile([128, D], F32, tag="bbc")
        nc.sync.dma_start(bias_bc[:], bias_f[None, :].broadcast_to([128, D]))
        yo = ctx.enter_context(tc.tile_pool(name="yo", bufs=3))
        for m in range(n_m):
            yp = att_ps.tile([128, D], F32, tag="st")
            for nch in range(2):
                sl = slice(nch * 512, (nch + 1) * 512)
                for kk in range(n_kk):
                    nc.tensor.matmul(
                        yp[:, sl],
                        lhsT=o_all[:, kk, m * 128:(m + 1) * 128],
                        rhs=wp_sb[:, kk, sl],
                        start=(kk == 0), stop=(kk == n_kk - 1))
            y_sb = yo.tile([128, D], F32, tag="y")
            nc.vector.tensor_tensor(y_sb[:], yp[:], bias_bc[:], OP.add)
            nc.sync.dma_start(y[m * 128:(m + 1) * 128, :], y_sb[:])

    nc.compile()
    return nc


# ---------------------------------------------------------------------------
# host-side sharding + entry point
# ---------------------------------------------------------------------------

_NC_CACHE = {}


def _get_nc(mode):
    if mode not in _NC_CACHE:
        _NC_CACHE[mode] = build_nc(mode)
    return _NC_CACHE[mode]


def _make_in_maps(x, Wq, Wk, Wv, Wp, bp, mode):
    x = np.asarray(x, dtype=np.float32)
    Wq = np.asarray(Wq, dtype=np.float32)
    Wk = np.asarray(Wk, dtype=np.float32)
    Wv = np.asarray(Wv, dtype=np.float32)
    Wp = np.asarray(Wp, dtype=np.float32)
    bp = np.asarray(bp, dtype=np.float32)

    adt_np = mybir.dt.np(ADT)
    mask = np.zeros((128, 128), dtype=np.float32)
    k_idx = np.arange(128)[:, None]
    q_idx = np.arange(128)[None, :]
    mask[q_idx >= k_idx] = 1.0
    mask = mask.astype(adt_np)

    xTs = [np.ascontiguousarray(x[b].T) for b in range(B)]
    in_maps = []
    for c in range(N_CORES):
        b, g = c // 2, c % 2
        rows = slice(512 * g, 512 * (g + 1))
        m = {
            "xT": xTs[b],
            "wqT": np.ascontiguousarray(Wq[rows, :].T),
            "wkT": np.ascontiguousarray(Wk[rows, :].T),
            "wvT": np.ascontiguousarray(Wv[rows, :].T),
            "mask": mask,
            "vone": np.ones((128, 128), dtype=adt_np),
        }
        if mode == "a2a":
            m["wpT"] = np.ascontiguousarray(Wp.T)
            m["bias"] = bp.reshape(1, D)
        else:
            m["wpT"] = np.ascontiguousarray(Wp[:, rows].T)
            m["bias"] = (bp if g == 0 else np.zeros_like(bp)).reshape(1, D)
        in_maps.append(m)
    return in_maps


def kernel(x, Wq, Wk, Wv, Wp, bp, _trace=False, _mode=None):
    mode = _mode or MODE
    nc = _get_nc(mode)
    in_maps = _make_in_maps(x, Wq, Wk, Wv, Wp, bp, mode)
    res = _run_spmd(nc, in_maps, trace=_trace)
    out = np.empty((B, T, D), dtype=np.float32)
    for b in range(B):
        if mode == "a2a":
            out[b, 0:QH] = res.results[2 * b]["y"]
            out[b, QH:T] = res.results[2 * b + 1]["y"]
        else:
            out[b] = res.results[2 * b]["y"] + res.results[2 * b + 1]["y"]
    if _trace:
        kernel.last_results = res
    return out


# revision 23
# speedup vs baseline: 1.1052x; 1.0032x over previous
"""Trainium2 Bass kernel for causal multi-head attention + output projection.

Problem (hardcoded): x[4, 2048, 1024] fp32, 16 heads, head_dim 64, causal,
torch-Linear convention (y = x @ W.T), output projection with bias.

Sharding over 8 NeuronCores: batch (4) x head-group (2 groups of 8 heads).
Each core computes q/k/v for its 8 heads of its batch, causal attention in
the S^T layout (keys on partitions, queries on free dim; softmax denominators
produced by an appended ones-column in V), then the output projection.

Combine modes:
  - "a2a": on-device AllToAll per head swaps query-halves between the two
    cores of a batch so each core projects all 16 heads for its own 1024
    queries; outputs are disjoint rows, host just concatenates.
  - "hostsum": each core emits a partial projection over its 8 heads for all
    2048 queries; host sums the pair (bias folded into group-0's input).

All matmuls run as float32r (TF32-like, ~1.5e-4 rel err, 4x faster than fp32).
"""
import os
import sys
import types

import numpy as np

import concourse.bass as bass
import concourse.mybir as mybir
import concourse.tile as tile
from concourse import bacc, bass_utils

DT = getattr(mybir.dt, os.environ.get("ATTN_DT_MAIN", "float32r"))
F32 = mybir.dt.float32
AF = mybir.ActivationFunctionType
OP = mybir.AluOpType

B, T, D = 4, 2048, 1024
H, HD = 16, 64
HG = 8          # heads per core
QH = T // 2     # query half
N_CORES = 8
SCALE = 1.0 / 8.0

MODE = os.environ.get("ATTN_KERNEL_MODE", "a2a")
ADT_NAME = os.environ.get("ATTN_DTYPE", "float32r")
ADT = getattr(mybir.dt, ADT_NAME)


# ---------------------------------------------------------------------------
# environment glue
# ---------------------------------------------------------------------------

def _install_ntff_hook():
    if 'antenv.axon_hooks' in sys.modules:
        return
    try:
        from trn_agent_boot.trn_boot import _ntff_profile_via_ctypes
        hook = _ntff_profile_via_ctypes('/opt/axon/libaxon_pjrt.so')
    except Exception:
        hook = None
    mod = types.ModuleType('antenv.axon_hooks')
    mod.get_axon_ntff_profile_hook = lambda: hook
    mod.set_axon_ntff_profile_hook = lambda h: None
    sys.modules['antenv.axon_hooks'] = mod


def _run_spmd(nc, in_maps, trace=False):
    from concourse.bass_interp import get_hw_module
    bass_utils.upload_artifacts = lambda tmpdir: tmpdir
    if trace:
        _install_ntff_hook()
    old_m = nc.m
    nc.m = get_hw_module(nc.m)
    try:
        return bass_utils.run_bass_kernel_spmd(
            nc, in_maps, core_ids=list(range(N_CORES)),
            trace=trace, trace_cores=[0] if trace else None,
        )
    finally:
        nc.m = old_m


# ---------------------------------------------------------------------------
# kernel program
# ---------------------------------------------------------------------------

def _qkv_phase(nc, tc, ctx, xT, wqT, wkT, wvT, vone, qT_sb, kT_sb, v_sb):
    """Compute q.T [512,2048], k.T [512,2048] and V' [2048, 8, 65] for this
    core's 8 heads. Contraction dim D lives on partitions; all operands fp32r."""
    xp = ctx.enter_context(tc.tile_pool(name="xph", bufs=16))
    wp = ctx.enter_context(tc.tile_pool(name="wph", bufs=12))
    ps = ctx.enter_context(tc.tile_pool(name="p2ps", bufs=2, space="PSUM"))

    xT_r = xT.rearrange("(ko ki) t -> ki ko t", ki=128)
    xh = [[None] * 8 for _ in range(2)]
    for kk in range(8):
        for half in range(2):
            t = xp.tile([128, QH], DT, tag="xh")
            nc.sync.dma_start(t[:], xT_r[:, kk, half * QH:(half + 1) * QH])
            xh[half][kk] = t

    def load_w(wT):
        parts = []
        wT_r = wT.rearrange("(ko ki) n -> ki ko n", ki=128)
        for kk in range(8):
            t = wp.tile([128, 512], DT, tag="w")
            nc.sync.dma_start(t[:], wT_r[:, kk])
            parts.append(t)
        return parts

    # k.T then V (both need full xT), then q.T half by half
    wk_sb = load_w(wkT)
    for m in range(4):
        for half in range(2):
            pt = ps.tile([128, QH], F32, tag="st")
            for nch in range(2):
                sl = slice(nch * 512, (nch + 1) * 512)
                for kk in range(8):
                    nc.tensor.matmul(
                        pt[:, sl],
                        lhsT=wk_sb[kk][:, m * 128:(m + 1) * 128],
                        rhs=xh[half][kk][:, sl],
                        start=(kk == 0), stop=(kk == 7))
            nc.vector.tensor_copy(kT_sb[:, m, half * QH:(half + 1) * QH], pt[:])

    wv_sb = load_w(wvT)
    nc.sync.dma_start(v_sb[:, :, :, 64],
                      vone.rearrange("p (a b) -> p a b", a=16))
    for m in range(16):
        pt = ps.tile([128, QH], F32, tag="st")
        for kk in range(8):
            nc.tensor.matmul(
                pt[:, 0:512],
                lhsT=xh[m // 8][kk][:, (m % 8) * 128:(m % 8 + 1) * 128],
                rhs=wv_sb[kk][:],
                start=(kk == 0), stop=(kk == 7))
        nc.vector.tensor_copy(
            v_sb[:, m, :, 0:64],
            pt[:, 0:512].rearrange("p (h d) -> p h d", h=HG))

    wq_sb = load_w(wqT)
    for half in range(2):
        for m in range(4):
            pt = ps.tile([128, QH], F32, tag="st")
            for nch in range(2):
                sl = slice(nch * 512, (nch + 1) * 512)
                for kk in range(8):
                    nc.tensor.matmul(
                        pt[:, sl],
                        lhsT=wq_sb[kk][:, m * 128:(m + 1) * 128],
                        rhs=xh[half][kk][:, sl],
                        start=(kk == 0), stop=(kk == 7))
            nc.vector.tensor_copy(qT_sb[:, m, half * QH:(half + 1) * QH], pt[:])


def _attend_pair(nc, p, qT_sb, kT_sb, v_sb, mask_sb, ps, es, snum, srec,
                 evict_cb, norm_cb):
    """Heads (2p, 2p+1) with their j-steps interleaved so the PE always has an
    independent S/AV matmul while the other head's exp runs on ACT. Rows 0..63
    of each accumulator are un-normalized O.T, row 64 the softmax denominators;
    normalization uses a reciprocal spread over 64 partitions via DRAM."""
    heads = (2 * p, 2 * p + 1)
    for qh in range(2):
        jmax = 8 * qh + 8
        o_ps = {h: ps.tile([65, QH], F32, tag="o", name=f"o{h}_{qh}")
                for h in heads}
        for j in range(jmax):
            qstart = max(QH * qh, 128 * j)
            n = QH * (qh + 1) - qstart
            coff = qstart - QH * qh
            e_sbs = {}
            for h in heads:
                pbase = 64 * (h % 2)
                sub = h // 2
                s_ps = ps.tile([128, QH], F32, tag="st", name=f"s{h}")
                for c in range(0, n, 512):
                    cn = min(512, n - c)
                    nc.tensor.matmul(
                        s_ps[:, c:c + cn],
                        lhsT=kT_sb[pbase:pbase + 64, sub, j * 128:(j + 1) * 128],
                        rhs=qT_sb[pbase:pbase + 64, sub,
                                  qstart + c:qstart + c + cn],
                        start=True, stop=True)
                e_sb = es.tile([128, QH], ADT, tag="es", name=f"e{h}")
                nc.scalar.activation(e_sb[:, 0:n], s_ps[:, 0:n], AF.Exp,
                                     scale=SCALE)
                if j >= 8 * qh:
                    nc.vector.tensor_tensor(
                        e_sb[:, 0:128], e_sb[:, 0:128], mask_sb[:], OP.mult)
                e_sbs[h] = e_sb
            for h in heads:
                c0 = coff
                while c0 < QH:
                    hi = min(QH, (c0 // 512 + 1) * 512)
                    nc.tensor.matmul(
                        o_ps[h][:, c0:hi],
                        lhsT=v_sb[:, j, h, :],
                        rhs=e_sbs[h][:, c0 - coff:hi - coff],
                        start=(j == 0), stop=(j == jmax - 1),
                        skip_group_check=True)
                    c0 = hi
        for h in heads:
            evict_cb(h, qh, o_ps[h])
            i = 4 * (h // 2) + 2 * (h % 2) + qh
            stmp = es.tile([1, QH], F32, tag="sr")
            nc.scalar.copy(stmp[:], o_ps[h][64:65, :])
            nc.sync.dma_start(snum[i:i + 1, :], stmp[:])
            st64 = es.tile([64, QH // 64], F32, tag="sp")
            nc.sync.dma_start(st64[:], snum[i].rearrange("(p f) -> p f", p=64))
            nc.vector.reciprocal(st64[:], st64[:])
            nc.sync.dma_start(srec[i].rearrange("(p f) -> p f", p=64), st64[:])
            bc = es.tile([128, QH], F32, tag="bc")
            nc.sync.dma_start(bc[:], srec[i][None, :].broadcast_to([128, QH]))
            norm_cb(h, qh, bc)


def build_nc(mode):
    nc = bacc.Bacc("TRN2", target_bir_lowering=False, debug=False,
                   enable_asserts=False, num_devices=N_CORES)
    xT = nc.dram_tensor("xT", [D, T], DT, kind="ExternalInput").ap()
    wqT = nc.dram_tensor("wqT", [D, 512], DT, kind="ExternalInput").ap()
    wkT = nc.dram_tensor("wkT", [D, 512], DT, kind="ExternalInput").ap()
    wvT = nc.dram_tensor("wvT", [D, 512], DT, kind="ExternalInput").ap()
    mask = nc.dram_tensor("mask", [128, 128], ADT, kind="ExternalInput").ap()
    vone = nc.dram_tensor("vone", [128, 128], ADT, kind="ExternalInput").ap()
    snum = nc.dram_tensor("snum", [16, QH], F32).ap()
    srec = nc.dram_tensor("srec", [16, QH], F32).ap()
    if mode == "a2a":
        wpT = nc.dram_tensor("wpT", [D, D], DT, kind="ExternalInput").ap()
        bias = nc.dram_tensor("bias", [1, D], F32, kind="ExternalInput").ap()
        y = nc.dram_tensor("y", [QH, D], F32, kind="ExternalOutput").ap()
    else:
        wpT = nc.dram_tensor("wpT", [512, D], DT, kind="ExternalInput").ap()
        bias = nc.dram_tensor("bias", [1, D], F32, kind="ExternalInput").ap()
        y = nc.dram_tensor("y", [T, D], F32, kind="ExternalOutput").ap()

    from contextlib import ExitStack
    with tile.TileContext(nc) as tc, ExitStack() as ctx:
        per = ctx.enter_context(tc.tile_pool(name="per", bufs=1))

        qT_sb = per.tile([128, 4, T], ADT, tag="qT")
        kT_sb = per.tile([128, 4, T], ADT, tag="kT")
        v_sb = per.tile([128, 16, HG, 65], ADT, tag="v")
        mask_sb = per.tile([128, 128], ADT, tag="mask")

        nc.sync.dma_start(mask_sb[:], mask[:])
        bias_f = bias[0]

        with ExitStack() as p2:
            _qkv_phase(nc, tc, p2, xT, wqT, wkT, wvT, vone, qT_sb, kT_sb, v_sb)

        # O accumulator (lives from attention through projection)
        mid = ctx.enter_context(tc.tile_pool(name="mid", bufs=1))
        o_all = mid.tile([128, 8 if mode == "a2a" else 4,
                          QH if mode == "a2a" else T], DT, tag="oacc")
        wp_sb = None
        if mode != "a2a":
            wp_sb = mid.tile([128, 4, D], DT, tag="wp")
            nc.sync.dma_start(wp_sb[:],
                              wpT.rearrange("(ko ki) n -> ki ko n", ki=128))

        att_ps = ctx.enter_context(tc.tile_pool(name="aps", bufs=2, space="PSUM"))
        with ExitStack() as attn:
            ps = att_ps
            es = attn.enter_context(tc.tile_pool(name="es", bufs=3))

            if mode == "a2a":
                dram = attn.enter_context(
                    tc.tile_pool(name="dram", bufs=2, space="DRAM"))
                oh_pool = attn.enter_context(tc.tile_pool(name="oh", bufs=2))
                for p in range(HG // 2):
                    ohs = {h: oh_pool.tile([64, T], DT, tag="oh", name=f"oh{h}")
                           for h in (2 * p, 2 * p + 1)}

                    def evict_cb(h, qh, o_ps, ohs=ohs):
                        nc.vector.tensor_copy(
                            ohs[h][:, QH * qh:QH * (qh + 1)], o_ps[0:64, :])

                    def norm_cb(h, qh, bc, ohs=ohs):
                        sl_ap = ohs[h][:, QH * qh:QH * (qh + 1)]
                        nc.vector.tensor_tensor(sl_ap, sl_ap, bc[0:64, :],
                                                OP.mult)

                    _attend_pair(nc, p, qT_sb, kT_sb, v_sb, mask_sb,
                                 ps, es, snum, srec, evict_cb, norm_cb)

                    for h in (2 * p, 2 * p + 1):
                        oh_sb = ohs[h]
                        in_b = dram.tile([2, 64, QH], DT, tag="cin")
                        out_b = dram.tile([2, 64, QH], DT, tag="cout",
                                          addr_space="Shared")
                        for s in range(2):
                            nc.sync.dma_start(in_b[s],
                                              oh_sb[:, s * QH:(s + 1) * QH])
                        nc.gpsimd.collective_compute(
                            "AllToAll", OP.bypass,
                            replica_groups=[[0, 1], [2, 3], [4, 5], [6, 7]],
                            ins=[in_b[:]], outs=[out_b[:]],
                        )
                        for s in range(2):
                            gh = 8 * s + h
                            nc.sync.dma_start(
                                o_all[64 * (gh % 2):64 * (gh % 2) + 64,
                                      gh // 2, :],
                                out_b[s])
            else:
                def evict_cb(h, qh, o_ps):
                    nc.vector.tensor_copy(
                        o_all[64 * (h % 2):64 * (h % 2) + 64, h // 2,
                              QH * qh:QH * (qh + 1)],
                        o_ps[0:64, :])

                def norm_cb(h, qh, bc):
                    pb = 64 * (h % 2)
                    sl_ap = o_all[pb:pb + 64, h // 2, QH * qh:QH * (qh + 1)]
                    nc.vector.tensor_tensor(sl_ap, sl_ap, bc[pb:pb + 64, :],
                                            OP.mult)

                for p in range(HG // 2):
                    _attend_pair(nc, p, qT_sb, kT_sb, v_sb, mask_sb,
                                 ps, es, snum, srec, evict_cb, norm_cb)

        # projection (psum from the attention pool so the scheduler can
        # overlap early m-tiles with the last head's attention)
        n_kk = 8 if mode == "a2a" else 4
        n_m = 8 if mode == "a2a" else 16
        if wp_sb is None:
            wp_sb = mid.tile([128, n_kk, D], DT, tag="wp")
            nc.sync.dma_start(
                wp_sb[:], wpT.rearrange("(ko ki) n -> ki ko n", ki=128))
        bias_bc = mid.tile([128, D], F32, tag="bbc")
        nc.sync.dma_start(bias_bc[:], bias_f[None, :].broadcast_to([128, D]))
        yo = ctx.enter_context(tc.tile_pool(name="yo", bufs=3))
        for m in range(n_m):
            yp = att_ps.tile([128, D], F32, tag="st")
            for nch in range(2):
                sl = slice(nch * 512, (nch + 1) * 512)
                for kk in range(n_kk):
                    nc.tensor.matmul(
                        yp[:, sl],
                        lhsT=o_all[:, kk, m * 128:(m + 1) * 128],
                        rhs=wp_sb[:, kk, sl],
                        start=(kk == 0), stop=(kk == n_kk - 1))
            y_sb = yo.tile([128, D], F32, tag="y")
            nc.vector.tensor_tensor(y_sb[:], yp[:], bias_bc[:], OP.add)
            nc.sync.dma_start(y[m * 128:(m + 1) * 128, :], y_sb[:])

    nc.compile()
    return nc


# ---------------------------------------------------------------------------
# host-side sharding + entry point
# ---------------------------------------------------------------------------

_NC_CACHE = {}


def _get_nc(mode):
    if mode not in _NC_CACHE:
        _NC_CACHE[mode] = build_nc(mode)
    return _NC_CACHE[mode]


def _make_in_maps(x, Wq, Wk, Wv, Wp, bp, mode):
    x = np.asarray(x, dtype=np.float32)
    Wq = np.asarray(Wq, dtype=np.float32)
    Wk = np.asarray(Wk, dtype=np.float32)
    Wv = np.asarray(Wv, dtype=np.float32)
    Wp = np.asarray(Wp, dtype=np.float32)
    bp = np.asarray(bp, dtype=np.float32)

    adt_np = mybir.dt.np(ADT)
    dt_np = mybir.dt.np(DT)
    mask = np.zeros((128, 128), dtype=np.float32)
    k_idx = np.arange(128)[:, None]
    q_idx = np.arange(128)[None, :]
    mask[q_idx >= k_idx] = 1.0
    mask = mask.astype(adt_np)

    xTs = [np.ascontiguousarray(x[b].T) for b in range(B)]
    in_maps = []
    for c in range(N_CORES):
        b, g = c // 2, c % 2
        rows = slice(512 * g, 512 * (g + 1))
        m = {
            "xT": xTs[b].astype(dt_np),
            "wqT": np.ascontiguousarray(Wq[rows, :].T).astype(dt_np),
            "wkT": np.ascontiguousarray(Wk[rows, :].T).astype(dt_np),
            "wvT": np.ascontiguousarray(Wv[rows, :].T).astype(dt_np),
            "mask": mask,
            "vone": np.ones((128, 128), dtype=adt_np),
        }
        if mode == "a2a":
            m["wpT"] = np.ascontiguousarray(Wp.T).astype(dt_np)
            m["bias"] = bp.reshape(1, D)
        else:
            m["wpT"] = np.ascontiguousarray(Wp[:, rows].T).astype(dt_np)
            m["bias"] = (bp if g == 0 else np.zeros_like(bp)).reshape(1, D)
        in_maps.append(m)
    return in_maps


def kernel(x, Wq, Wk, Wv, Wp, bp, _trace=False, _mode=None):
    mode = _mode or MODE
    nc = _get_nc(mode)
    in_maps = _make_in_maps(x, Wq, Wk, Wv, Wp, bp, mode)
    res = _run_spmd(nc, in_maps, trace=_trace)
    out = np.empty((B, T, D), dtype=np.float32)
    for b in range(B):
        if mode == "a2a":
            out[b, 0:QH] = res.results[2 * b]["y"]
            out[b, QH:T] = res.results[2 * b + 1]["y"]
        else:
            out[b] = res.results[2 * b]["y"] + res.results[2 * b + 1]["y"]
    if _trace:
        kernel.last_results = res
    return out


# revision 24
# speedup vs baseline: 1.1123x; 1.0065x over previous
"""Trainium2 Bass kernel for causal multi-head attention + output projection.

Problem (hardcoded): x[4, 2048, 1024] fp32, 16 heads, head_dim 64, causal,
torch-Linear convention (y = x @ W.T), output projection with bias.

Sharding over 8 NeuronCores: batch (4) x head-group (2 groups of 8 heads).
Each core computes q/k/v for its 8 heads of its batch, causal attention in
the S^T layout (keys on partitions, queries on free dim; softmax denominators
produced by an appended ones-column in V), then the output projection.

Combine modes:
  - "a2a": on-device AllToAll per head swaps query-halves between the two
    cores of a batch so each core projects all 16 heads for its own 1024
    queries; outputs are disjoint rows, host just concatenates.
  - "hostsum": each core emits a partial projection over its 8 heads for all
    2048 queries; host sums the pair (bias folded into group-0's input).

All matmuls run as float32r (TF32-like, ~1.5e-4 rel err, 4x faster than fp32).
"""
import os
import sys
import types

import numpy as np

import concourse.bass as bass
import concourse.mybir as mybir
import concourse.tile as tile
from concourse import bacc, bass_utils

DT = getattr(mybir.dt, os.environ.get("ATTN_DT_MAIN", "float32r"))
F32 = mybir.dt.float32
AF = mybir.ActivationFunctionType
OP = mybir.AluOpType

B, T, D = 4, 2048, 1024
H, HD = 16, 64
HG = 8          # heads per core
QH = T // 2     # query half
N_CORES = 8
SCALE = 1.0 / 8.0

MODE = os.environ.get("ATTN_KERNEL_MODE", "a2a")
ADT_NAME = os.environ.get("ATTN_DTYPE", "float32r")
ADT = getattr(mybir.dt, ADT_NAME)


# ---------------------------------------------------------------------------
# environment glue
# ---------------------------------------------------------------------------

def _install_ntff_hook():
    if 'antenv.axon_hooks' in sys.modules:
        return
    try:
        from trn_agent_boot.trn_boot import _ntff_profile_via_ctypes
        hook = _ntff_profile_via_ctypes('/opt/axon/libaxon_pjrt.so')
    except Exception:
        hook = None
    mod = types.ModuleType('antenv.axon_hooks')
    mod.get_axon_ntff_profile_hook = lambda: hook
    mod.set_axon_ntff_profile_hook = lambda h: None
    sys.modules['antenv.axon_hooks'] = mod


def _run_spmd(nc, in_maps, trace=False):
    from concourse.bass_interp import get_hw_module
    bass_utils.upload_artifacts = lambda tmpdir: tmpdir
    if trace:
        _install_ntff_hook()
    old_m = nc.m
    nc.m = get_hw_module(nc.m)
    try:
        return bass_utils.run_bass_kernel_spmd(
            nc, in_maps, core_ids=list(range(N_CORES)),
            trace=trace, trace_cores=[0] if trace else None,
        )
    finally:
        nc.m = old_m


# ---------------------------------------------------------------------------
# kernel program
# ---------------------------------------------------------------------------

def _qkv_phase(nc, tc, ctx, xT, wqT, wkT, wvT, vone, qT_sb, kT_sb, v_sb):
    """Compute q.T [512,2048], k.T [512,2048] and V' [2048, 8, 65] for this
    core's 8 heads. Contraction dim D lives on partitions; all operands fp32r."""
    xp = ctx.enter_context(tc.tile_pool(name="xph", bufs=16))
    wp = ctx.enter_context(tc.tile_pool(name="wph", bufs=12))
    ps = ctx.enter_context(tc.tile_pool(name="p2ps", bufs=2, space="PSUM"))

    xT_r = xT.rearrange("(ko ki) t -> ki ko t", ki=128)

    def load_w(wT):
        parts = []
        wT_r = wT.rearrange("(ko ki) n -> ki ko n", ki=128)
        for kk in range(8):
            t = wp.tile([128, 512], DT, tag="w")
            nc.sync.dma_start(t[:], wT_r[:, kk])
            parts.append(t)
        return parts

    # k.T first: emit its weight slices and the x slices in consumption order
    # so the first matmuls start as soon as ~1MB has landed.
    wk_sb = load_w(wkT)
    xh = [[None] * 8 for _ in range(2)]
    for kk in range(8):
        for half in range(2):
            t = xp.tile([128, QH], DT, tag="xh")
            nc.sync.dma_start(t[:], xT_r[:, kk, half * QH:(half + 1) * QH])
            xh[half][kk] = t
    for m in range(4):
        for half in range(2):
            pt = ps.tile([128, QH], F32, tag="st")
            for nch in range(2):
                sl = slice(nch * 512, (nch + 1) * 512)
                for kk in range(8):
                    nc.tensor.matmul(
                        pt[:, sl],
                        lhsT=wk_sb[kk][:, m * 128:(m + 1) * 128],
                        rhs=xh[half][kk][:, sl],
                        start=(kk == 0), stop=(kk == 7))
            nc.vector.tensor_copy(kT_sb[:, m, half * QH:(half + 1) * QH], pt[:])

    wv_sb = load_w(wvT)
    nc.sync.dma_start(v_sb[:, :, :, 64],
                      vone.rearrange("p (a b) -> p a b", a=16))
    for m in range(16):
        pt = ps.tile([128, QH], F32, tag="st")
        for kk in range(8):
            nc.tensor.matmul(
                pt[:, 0:512],
                lhsT=xh[m // 8][kk][:, (m % 8) * 128:(m % 8 + 1) * 128],
                rhs=wv_sb[kk][:],
                start=(kk == 0), stop=(kk == 7))
        nc.vector.tensor_copy(
            v_sb[:, m, :, 0:64],
            pt[:, 0:512].rearrange("p (h d) -> p h d", h=HG))

    wq_sb = load_w(wqT)
    for half in range(2):
        for m in range(4):
            pt = ps.tile([128, QH], F32, tag="st")
            for nch in range(2):
                sl = slice(nch * 512, (nch + 1) * 512)
                for kk in range(8):
                    nc.tensor.matmul(
                        pt[:, sl],
                        lhsT=wq_sb[kk][:, m * 128:(m + 1) * 128],
                        rhs=xh[half][kk][:, sl],
                        start=(kk == 0), stop=(kk == 7))
            nc.vector.tensor_copy(qT_sb[:, m, half * QH:(half + 1) * QH], pt[:])


def _attend_pair(nc, p, qT_sb, kT_sb, v_sb, mask_sb, ps, es, snum, srec,
                 evict_cb, norm_cb):
    """Heads (2p, 2p+1) with their j-steps interleaved so the PE always has an
    independent S/AV matmul while the other head's exp runs on ACT. Rows 0..63
    of each accumulator are un-normalized O.T, row 64 the softmax denominators;
    normalization uses a reciprocal spread over 64 partitions via DRAM."""
    heads = (2 * p, 2 * p + 1)
    for qh in range(2):
        jmax = 8 * qh + 8
        o_ps = {h: ps.tile([65, QH], F32, tag="o", name=f"o{h}_{qh}")
                for h in heads}
        for j in range(jmax):
            qstart = max(QH * qh, 128 * j)
            n = QH * (qh + 1) - qstart
            coff = qstart - QH * qh
            e_sbs = {}
            for h in heads:
                pbase = 64 * (h % 2)
                sub = h // 2
                s_ps = ps.tile([128, QH], F32, tag="st", name=f"s{h}")
                for c in range(0, n, 512):
                    cn = min(512, n - c)
                    nc.tensor.matmul(
                        s_ps[:, c:c + cn],
                        lhsT=kT_sb[pbase:pbase + 64, sub, j * 128:(j + 1) * 128],
                        rhs=qT_sb[pbase:pbase + 64, sub,
                                  qstart + c:qstart + c + cn],
                        start=True, stop=True)
                e_sb = es.tile([128, QH], ADT, tag="es", name=f"e{h}")
                nc.scalar.activation(e_sb[:, 0:n], s_ps[:, 0:n], AF.Exp,
                                     scale=SCALE)
                if j >= 8 * qh:
                    nc.vector.tensor_tensor(
                        e_sb[:, 0:128], e_sb[:, 0:128], mask_sb[:], OP.mult)
                e_sbs[h] = e_sb
            for h in heads:
                c0 = coff
                while c0 < QH:
                    hi = min(QH, (c0 // 512 + 1) * 512)
                    nc.tensor.matmul(
                        o_ps[h][:, c0:hi],
                        lhsT=v_sb[:, j, h, :],
                        rhs=e_sbs[h][:, c0 - coff:hi - coff],
                        start=(j == 0), stop=(j == jmax - 1),
                        skip_group_check=True)
                    c0 = hi
        for h in heads:
            evict_cb(h, qh, o_ps[h])
            i = 4 * (h // 2) + 2 * (h % 2) + qh
            stmp = es.tile([1, QH], F32, tag="sr")
            nc.scalar.copy(stmp[:], o_ps[h][64:65, :])
            nc.sync.dma_start(snum[i:i + 1, :], stmp[:])
            st64 = es.tile([64, QH // 64], F32, tag="sp")
            nc.sync.dma_start(st64[:], snum[i].rearrange("(p f) -> p f", p=64))
            nc.vector.reciprocal(st64[:], st64[:])
            nc.sync.dma_start(srec[i].rearrange("(p f) -> p f", p=64), st64[:])
            bc = es.tile([128, QH], F32, tag="bc")
            nc.sync.dma_start(bc[:], srec[i][None, :].broadcast_to([128, QH]))
            norm_cb(h, qh, bc)


def build_nc(mode):
    nc = bacc.Bacc("TRN2", target_bir_lowering=False, debug=False,
                   enable_asserts=False, num_devices=N_CORES)
    xT = nc.dram_tensor("xT", [D, T], DT, kind="ExternalInput").ap()
    wqT = nc.dram_tensor("wqT", [D, 512], DT, kind="ExternalInput").ap()
    wkT = nc.dram_tensor("wkT", [D, 512], DT, kind="ExternalInput").ap()
    wvT = nc.dram_tensor("wvT", [D, 512], DT, kind="ExternalInput").ap()
    mask = nc.dram_tensor("mask", [128, 128], ADT, kind="ExternalInput").ap()
    vone = nc.dram_tensor("vone", [128, 128], ADT, kind="ExternalInput").ap()
    snum = nc.dram_tensor("snum", [16, QH], F32).ap()
    srec = nc.dram_tensor("srec", [16, QH], F32).ap()
    if mode == "a2a":
        wpT = nc.dram_tensor("wpT", [D, D], DT, kind="ExternalInput").ap()
        bias = nc.dram_tensor("bias", [1, D], F32, kind="ExternalInput").ap()
        y = nc.dram_tensor("y", [QH, D], F32, kind="ExternalOutput").ap()
    else:
        wpT = nc.dram_tensor("wpT", [512, D], DT, kind="ExternalInput").ap()
        bias = nc.dram_tensor("bias", [1, D], F32, kind="ExternalInput").ap()
        y = nc.dram_tensor("y", [T, D], F32, kind="ExternalOutput").ap()

    from contextlib import ExitStack
    with tile.TileContext(nc) as tc, ExitStack() as ctx:
        per = ctx.enter_context(tc.tile_pool(name="per", bufs=1))

        qT_sb = per.tile([128, 4, T], ADT, tag="qT")
        kT_sb = per.tile([128, 4, T], ADT, tag="kT")
        v_sb = per.tile([128, 16, HG, 65], ADT, tag="v")
        mask_sb = per.tile([128, 128], ADT, tag="mask")

        nc.sync.dma_start(mask_sb[:], mask[:])
        bias_f = bias[0]

        with ExitStack() as p2:
            _qkv_phase(nc, tc, p2, xT, wqT, wkT, wvT, vone, qT_sb, kT_sb, v_sb)

        # O accumulator (lives from attention through projection)
        mid = ctx.enter_context(tc.tile_pool(name="mid", bufs=1))
        o_all = mid.tile([128, 8 if mode == "a2a" else 4,
                          QH if mode == "a2a" else T], DT, tag="oacc")
        wp_sb = None
        if mode != "a2a":
            wp_sb = mid.tile([128, 4, D], DT, tag="wp")
            nc.sync.dma_start(wp_sb[:],
                              wpT.rearrange("(ko ki) n -> ki ko n", ki=128))

        att_ps = ctx.enter_context(tc.tile_pool(name="aps", bufs=2, space="PSUM"))
        with ExitStack() as attn:
            ps = att_ps
            es = attn.enter_context(tc.tile_pool(name="es", bufs=3))

            if mode == "a2a":
                dram = attn.enter_context(
                    tc.tile_pool(name="dram", bufs=2, space="DRAM"))
                oh_pool = attn.enter_context(tc.tile_pool(name="oh", bufs=2))
                for p in range(HG // 2):
                    ohs = {h: oh_pool.tile([64, T], DT, tag="oh", name=f"oh{h}")
                           for h in (2 * p, 2 * p + 1)}

                    def evict_cb(h, qh, o_ps, ohs=ohs):
                        nc.vector.tensor_copy(
                            ohs[h][:, QH * qh:QH * (qh + 1)], o_ps[0:64, :])

                    def norm_cb(h, qh, bc, ohs=ohs):
                        sl_ap = ohs[h][:, QH * qh:QH * (qh + 1)]
                        nc.vector.tensor_tensor(sl_ap, sl_ap, bc[0:64, :],
                                                OP.mult)

                    _attend_pair(nc, p, qT_sb, kT_sb, v_sb, mask_sb,
                                 ps, es, snum, srec, evict_cb, norm_cb)

                    for h in (2 * p, 2 * p + 1):
                        oh_sb = ohs[h]
                        in_b = dram.tile([2, 64, QH], DT, tag="cin")
                        out_b = dram.tile([2, 64, QH], DT, tag="cout",
                                          addr_space="Shared")
                        for s in range(2):
                            nc.sync.dma_start(in_b[s],
                                              oh_sb[:, s * QH:(s + 1) * QH])
                        nc.gpsimd.collective_compute(
                            "AllToAll", OP.bypass,
                            replica_groups=[[0, 1], [2, 3], [4, 5], [6, 7]],
                            ins=[in_b[:]], outs=[out_b[:]],
                        )
                        for s in range(2):
                            gh = 8 * s + h
                            nc.sync.dma_start(
                                o_all[64 * (gh % 2):64 * (gh % 2) + 64,
                                      gh // 2, :],
                                out_b[s])
            else:
                def evict_cb(h, qh, o_ps):
                    nc.vector.tensor_copy(
                        o_all[64 * (h % 2):64 * (h % 2) + 64, h // 2,
                              QH * qh:QH * (qh + 1)],
                        o_ps[0:64, :])

                def norm_cb(h, qh, bc):
                    pb = 64 * (h % 2)
                    sl_ap = o_all[pb:pb + 64, h // 2, QH * qh:QH * (qh + 1)]
                    nc.vector.tensor_tensor(sl_ap, sl_ap, bc[pb:pb + 64, :],
                                            OP.mult)

                for p in range(HG // 2):
                    _attend_pair(nc, p, qT_sb, kT_sb, v_sb, mask_sb,
                                 ps, es, snum, srec, evict_cb, norm_cb)

        # projection (psum from the attention pool so the scheduler can
        # overlap early m-tiles with the last head's attention)
        n_kk = 8 if mode == "a2a" else 4
        n_m = 8 if mode == "a2a" else 16
        if wp_sb is None:
            wp_sb = mid.tile([128, n_kk, D], DT, tag="wp")
            nc.sync.dma_start(
                wp_sb[:], wpT.rearrange("(ko ki) n -> ki ko n", ki=128))
        bias_bc = mid.tile([128, D], F32, tag="bbc")
        nc.sync.dma_start(bias_bc[:], bias_f[None, :].broadcast_to([128, D]))
        yo = ctx.enter_context(tc.tile_pool(name="yo", bufs=3))
        for m in range(n_m):
            yp = att_ps.tile([128, D], F32, tag="st")
            for nch in range(2):
                sl = slice(nch * 512, (nch + 1) * 512)
                for kk in range(n_kk):
                    nc.tensor.matmul(
                        yp[:, sl],
                        lhsT=o_all[:, kk, m * 128:(m + 1) * 128],
                        rhs=wp_sb[:, kk, sl],
                        start=(kk == 0), stop=(kk == n_kk - 1))
            y_sb = yo.tile([128, D], F32, tag="y")
            nc.vector.tensor_tensor(y_sb[:], yp[:], bias_bc[:], OP.add)
            nc.sync.dma_start(y[m * 128:(m + 1) * 128, :], y_sb[:])

    nc.compile()
    return nc


# ---------------------------------------------------------------------------
# host-side sharding + entry point
# ---------------------------------------------------------------------------

_NC_CACHE = {}


def _get_nc(mode):
    if mode not in _NC_CACHE:
        _NC_CACHE[mode] = build_nc(mode)
    return _NC_CACHE[mode]


def _make_in_maps(x, Wq, Wk, Wv, Wp, bp, mode):
    x = np.asarray(x, dtype=np.float32)
    Wq = np.asarray(Wq, dtype=np.float32)
    Wk = np.asarray(Wk, dtype=np.float32)
    Wv = np.asarray(Wv, dtype=np.float32)
    Wp = np.asarray(Wp, dtype=np.float32)
    bp = np.asarray(bp, dtype=np.float32)

    adt_np = mybir.dt.np(ADT)
    dt_np = mybir.dt.np(DT)
    mask = np.zeros((128, 128), dtype=np.float32)
    k_idx = np.arange(128)[:, None]
    q_idx = np.arange(128)[None, :]
    mask[q_idx >= k_idx] = 1.0
    mask = mask.astype(adt_np)

    xTs = [np.ascontiguousarray(x[b].T) for b in range(B)]
    in_maps = []
    for c in range(N_CORES):
        b, g = c // 2, c % 2
        rows = slice(512 * g, 512 * (g + 1))
        m = {
            "xT": xTs[b].astype(dt_np),
            "wqT": np.ascontiguousarray(Wq[rows, :].T).astype(dt_np),
            "wkT": np.ascontiguousarray(Wk[rows, :].T).astype(dt_np),
            "wvT": np.ascontiguousarray(Wv[rows, :].T).astype(dt_np),
            "mask": mask,
            "vone": np.ones((128, 128), dtype=adt_np),
        }
        if mode == "a2a":
            m["wpT"] = np.ascontiguousarray(Wp.T).astype(dt_np)
            m["bias"] = bp.reshape(1, D)
        else:
            m["wpT"] = np.ascontiguousarray(Wp[:, rows].T).astype(dt_np)
            m["bias"] = (bp if g == 0 else np.zeros_like(bp)).reshape(1, D)
        in_maps.append(m)
    return in_maps


def kernel(x, Wq, Wk, Wv, Wp, bp, _trace=False, _mode=None):
    mode = _mode or MODE
    nc = _get_nc(mode)
    in_maps = _make_in_maps(x, Wq, Wk, Wv, Wp, bp, mode)
    res = _run_spmd(nc, in_maps, trace=_trace)
    out = np.empty((B, T, D), dtype=np.float32)
    for b in range(B):
        if mode == "a2a":
            out[b, 0:QH] = res.results[2 * b]["y"]
            out[b, QH:T] = res.results[2 * b + 1]["y"]
        else:
            out[b] = res.results[2 * b]["y"] + res.results[2 * b + 1]["y"]
    if _trace:
        kernel.last_results = res
    return out
